# revision 2
# baseline (speedup 1.0000x reference)
"""KPConv-style GNN message passing on 8 TRN2 NeuronCores.

out[m, o] = sum_{e: target[e]=m} sum_i w[o, k_e, i] * features[source[e], i]
k_e = argmin_k ||hood_coords[e] - mu[k]||^2

Sharding: nodes are partitioned into 8 contiguous spans of 3125 (one per
core); each edge is routed to the core owning its target node, so no
cross-core reduction is needed. Within a core, edges are bucketed into 25
windows of 125 nodes and padded to a fixed 2304 edges per window (18 chunks
of 128). Per chunk the device computes nearest-kernel-point scores with a
tiny matmul (f32), all-K per-edge outputs Y = F^T @ Wflat (bf16, PSUM f32),
collapses K via an is_equal mask + add tree on DVE, and scatter-adds into
the 125-node window with a one-hot matmul accumulated in PSUM.
"""

import numpy as np
import ml_dtypes

E_TOT = 400000
M_NODES = 25000
FI = 32          # input features
FO = 32          # output features
KPTS = 15
KP = 16          # padded K
NCORES = 8
M_CORE = 3125    # nodes per core
WIN_NODES = 125  # nodes per window
N_WIN = 25       # windows per core
E_WIN = 2304     # padded edges per window
N_CHUNK = E_WIN // 128
E_PAD = N_WIN * E_WIN
PAD_COL = 126    # one-hot column for padding edges (row never stored)

_CACHE = {}


def _build_nc(n_win=N_WIN):
    from concourse import bacc, mybir, tile
    from concourse import library_config

    f32 = mybir.dt.float32
    bf16 = mybir.dt.bfloat16
    i16 = mybir.dt.int16
    eq = mybir.AluOpType.is_equal
    mult = mybir.AluOpType.mult
    add = mybir.AluOpType.add

    nc = bacc.Bacc("TRN2", target_bir_lowering=False, debug=False)

    feat = nc.declare_dram_parameter("feat", [M_NODES, 128], bf16, isOutput=False)
    e_pad = n_win * E_WIN
    hoodT = nc.declare_dram_parameter("hoodT", [4, e_pad], f32, isOutput=False)
    srcidx = nc.declare_dram_parameter("srcidx", [128, e_pad // 16], i16, isOutput=False)
    tgtw = nc.declare_dram_parameter("tgtw", [n_win * 128, N_CHUNK], bf16, isOutput=False)
    wflat = nc.declare_dram_parameter("wflat", [FI, KP * FO], bf16, isOutput=False)
    muaug = nc.declare_dram_parameter("muaug", [4, KP], f32, isOutput=False)
    iota = nc.declare_dram_parameter("iota", [128, 128], bf16, isOutput=False)
    out = nc.declare_dram_parameter("out", [n_win * WIN_NODES, FO], f32, isOutput=True)

    with tile.TileContext(nc) as tc:
        with (
            tc.tile_pool(name="const", bufs=1) as cpool,
            tc.tile_pool(name="win", bufs=2) as wpool,
            tc.tile_pool(name="chunk", bufs=4) as kpool,
            tc.tile_pool(name="ps", bufs=2, space="PSUM") as ppool,
            tc.tile_pool(name="pso", bufs=2, space="PSUM") as opool,
        ):
            with tc.tile_critical():
                nc.gpsimd.load_library(library_config.mlp)

            wflat_sb = cpool.tile([FI, KP * FO], bf16, tag="wflat")
            muaug_sb = cpool.tile([4, KP], f32, tag="muaug")
            iota_sb = cpool.tile([128, 128], bf16, tag="iota")
            srcidx_sb = cpool.tile([128, e_pad // 16], i16, tag="srcidx")
            nc.sync.dma_start(wflat_sb[:], wflat[:])
            nc.sync.dma_start(muaug_sb[:], muaug[:])
            nc.sync.dma_start(iota_sb[:], iota[:])
            nc.sync.dma_start(srcidx_sb[:], srcidx[:])

            for w in range(n_win):
                ftile = wpool.tile([128, 1, E_WIN], bf16, tag="ftile")
                htile = wpool.tile([4, E_WIN], f32, tag="htile")
                ttile = wpool.tile([128, N_CHUNK], bf16, tag="ttile")

                GSUB = 384
                for g in range(E_WIN // GSUB):
                    nc.gpsimd.dma_gather(
                        ftile[:, :, g * GSUB:(g + 1) * GSUB],
                        feat[:],
                        srcidx_sb[:, (w * E_WIN + g * GSUB) // 16:
                                  (w * E_WIN + (g + 1) * GSUB) // 16],
                        GSUB,
                        GSUB,
                        128,
                        transpose=True,
                    )
                nc.sync.dma_start(htile[:], hoodT[:, w * E_WIN:(w + 1) * E_WIN])
                nc.sync.dma_start(ttile[:], tgtw[w * 128:(w + 1) * 128, :])

                ps_o = opool.tile([128, FO], f32, tag="ps_o")

                for c in range(N_CHUNK):
                    lo = c * 128

                    # nearest kernel point scores: [128e, 16k] f32
                    ps_s = ppool.tile([128, KP], f32, tag="ps_s")
                    nc.tensor.matmul(
                        ps_s[:], htile[:, lo:lo + 128], muaug_sb[:],
                        start=True, stop=True,
                    )
                    ssb = kpool.tile([128, KP], f32, tag="ssb")
                    nc.scalar.activation(ssb[:], ps_s[:], mybir.ActivationFunctionType.Copy)
                    m8 = kpool.tile([128, 8], f32, tag="m8")
                    nc.vector.max(m8[:], ssb[:])
                    mask = kpool.tile([128, KP], bf16, tag="mask")
                    nc.vector.tensor_tensor(
                        out=mask[:], in0=ssb[:], in1=m8[:, 0:1].broadcast_to([128, KP]), op=eq,
                    )

                    # all-K edge outputs: Y[128e, 512] = F^T chunk.T @ Wflat
                    ps_y = ppool.tile([128, KP * FO], f32, tag="ps_y")
                    nc.tensor.matmul(
                        ps_y[:], ftile[0:FI, 0, lo:lo + 128], wflat_sb[:],
                        start=True, stop=True,
                    )
                    y3 = kpool.tile([128, KP * FO], bf16, tag="y3")
                    nc.scalar.activation(y3[:], ps_y[:], mybir.ActivationFunctionType.Copy)

                    # K-collapse: mask-mul then contiguous add tree (k-major)
                    my = kpool.tile([128, KP * FO], bf16, tag="my")
                    nc.vector.tensor_tensor(
                        out=my[:].rearrange("p (k o) -> p k o", k=KP),
                        in0=y3[:].rearrange("p (k o) -> p k o", k=KP),
                        in1=mask[:].rearrange("p (k o) -> p k o", o=1).broadcast_to([128, KP, FO]),
                        op=mult,
                    )
                    t1 = kpool.tile([128, 256], bf16, tag="t1")
                    nc.vector.tensor_tensor(
                        out=t1[:], in0=my[:, 0:256], in1=my[:, 256:512], op=add)
                    t2 = kpool.tile([128, 128], bf16, tag="t2")
                    nc.vector.tensor_tensor(
                        out=t2[:], in0=t1[:, 0:128], in1=t1[:, 128:256], op=add)
                    t3 = kpool.tile([128, 64], bf16, tag="t3")
                    nc.vector.tensor_tensor(
                        out=t3[:], in0=t2[:, 0:64], in1=t2[:, 64:128], op=add)
                    ye = kpool.tile([128, FO], bf16, tag="ye")
                    nc.vector.tensor_tensor(
                        out=ye[:], in0=t3[:, 0:FO], in1=t3[:, FO:64], op=add)

                    # one-hot of window-local target and scatter via PE
                    oh = kpool.tile([128, 128], bf16, tag="oh")
                    nc.vector.tensor_tensor(
                        out=oh[:], in0=ttile[:, c:c + 1].broadcast_to([128, 128]),
                        in1=iota_sb[:], op=eq,
                    )
                    nc.tensor.matmul(
                        ps_o[:], oh[:], ye[:],
                        start=(c == 0), stop=(c == N_CHUNK - 1),
                    )

                osb = kpool.tile([128, FO], f32, tag="osb")
                nc.scalar.activation(osb[:], ps_o[:], mybir.ActivationFunctionType.Copy)
                nc.sync.dma_start(
                    out[w * WIN_NODES:(w + 1) * WIN_NODES, :], osb[0:WIN_NODES, :])

    nc.compile()
    return nc


def _host_prep(source, target, features, hood_coords, mu, w,
               n_win=N_WIN, m_core=M_CORE, ncores=NCORES):
    bf = ml_dtypes.bfloat16
    src = np.ascontiguousarray(source.astype(np.int64))
    tgt = np.ascontiguousarray(target.astype(np.int64))

    feat = np.zeros((M_NODES, 128), dtype=bf)
    feat[:, :FI] = features.astype(bf)

    wfl = np.zeros((FI, KP * FO), dtype=bf)
    # wflat[i, 32k+o] = w[o,k,i]
    wfl[:, :KPTS * FO] = np.transpose(w, (2, 1, 0)).reshape(FI, KPTS * FO).astype(bf)

    mu0 = mu[0].astype(np.float64)  # [15, 3]
    mua = np.zeros((4, KP), dtype=np.float32)
    mua[0:3, :KPTS] = (2.0 * mu0.T).astype(np.float32)
    mua[3, :KPTS] = (-np.sum(mu0 * mu0, axis=1)).astype(np.float32)
    mua[3, KPTS:] = -1e30

    iota = np.broadcast_to(np.arange(128, dtype=np.float32), (128, 128)).astype(bf)
    iota = np.ascontiguousarray(iota)

    # route edges to cores by target ownership, then to windows
    e_pad = n_win * E_WIN
    core_of = tgt // m_core
    local = tgt - core_of * m_core
    win_of = local // WIN_NODES
    col_of = local - win_of * WIN_NODES  # in [0, 125)

    in_maps = []
    order = np.argsort(core_of * n_win + win_of, kind="stable")
    bucket_ids = (core_of * n_win + win_of)[order]
    bounds = np.searchsorted(bucket_ids, np.arange(ncores * n_win + 1))

    for cid in range(ncores):
        hood_p = np.zeros((e_pad, 3), dtype=np.float32)
        src_p = np.zeros(e_pad, dtype=np.int64)
        col_p = np.full(e_pad, PAD_COL, dtype=np.float32)
        for wi in range(n_win):
            b = cid * n_win + wi
            sel = order[bounds[b]:bounds[b + 1]]
            n = len(sel)
            if n > E_WIN:
                raise RuntimeError(f"window overflow: {n} > {E_WIN}")
            base = wi * E_WIN
            hood_p[base:base + n] = hood_coords[sel]
            src_p[base:base + n] = src[sel]
            col_p[base:base + n] = col_of[sel]

        hoodT = np.empty((4, e_pad), dtype=np.float32)
        hoodT[0:3] = hood_p.T
        hoodT[3] = 1.0

        # gather idx layout: [128, E_PAD//16], idx j at [j%16 + 16*r, j//16] for all r
        si = src_p.astype(np.int16).reshape(e_pad // 16, 16).T
        srcidx = np.ascontiguousarray(np.tile(si, (8, 1)))

        # tgtw[w*128+p, c] = col of edge (w, c*128+p)
        tw = col_p.reshape(n_win, N_CHUNK, 128).transpose(0, 2, 1).reshape(
            n_win * 128, N_CHUNK).astype(bf)

        in_maps.append({
            "feat": feat,
            "hoodT": hoodT,
            "srcidx": srcidx,
            "tgtw": np.ascontiguousarray(tw),
            "wflat": wfl,
            "muaug": mua,
            "iota": iota,
        })
    return in_maps


def kernel(source, target, features, hood_coords, mu, w):
    from concourse.bass_utils import run_bass_kernel_spmd

    if "nc" not in _CACHE:
        _CACHE["nc"] = _build_nc()
    nc = _CACHE["nc"]

    in_maps = _host_prep(source, target, features, hood_coords, mu, w)
    res = run_bass_kernel_spmd(nc, in_maps, list(range(NCORES)))
    _CACHE["last_result"] = res
    parts = [res.results[c]["out"] for c in range(NCORES)]
    return np.concatenate(parts, axis=0).astype(np.float32)

